# revision 33
# baseline (speedup 1.0000x reference)
"""AConnect (nn_AConnect_82368882803074) Trainium2 kernel, v2.

Reference computation:
    memW[b]    = W * Werr_bank[idx[b]]             [B, D_in, D_out]
    membias[b] = bias * Berr_bank[idx[b]]          [B, 1, D_out]
    Z[b]       = X[b] @ memW[b] + membias[b]       [B, D_out]

Strategy: data-parallel over the batch across 8 NeuronCores with global
bank dedup. The host groups samples by bank index and packs one bank per
"slot" (up to M=4 samples ride along as extra matmul columns); slots are
spread over the 8 cores. The host only moves/casts data (gather,
transpose, bf16 cast, zero-padding, output permutation); all arithmetic
(W ⊙ E, X @ (W ⊙ E), bias ⊙ Berr and the final add) runs on device.

Changes vs v1 (v1: 94.9 us harness / 111.8 us traced; now ~75-82 us traced,
run-to-run HAM-phase variance is ~+/-4 us):
- Banks are cast to bf16 on the host, halving HBM read traffic (the
  dominant cost) from ~29 MB to ~15 MB per core; W/X upload bf16 too, so
  no device-side casts remain and SWDGE cast-DMA (1.8 us/transfer gpsimd
  ucode, ~5 us engine-library warmup) is not needed.
- Bank loads are paired (1 MB per DMA) and alternate between the two
  HWDGE rings (sync + scalar engines), each ring moving ~360 GB/s; the
  gpsimd SWDGE ring measured ~6 us between issues and is not used.
- Per-slot PSUM drains (ScalarE, [4, 1024] per slot pair) write one
  staging tile; a single rearranging store at the end replaces v1's
  per-pair output DMAs.
- VectorE runs exactly one [128, 2048] bf16 multiply per slot (2x mode,
  ~1.22 us) and nothing else — membias moved to gpsimd, W-doubling and
  all casts moved to the host.
- The per-slot k=1 bias matmul is kept even for zero bias: removing it
  measured ~5 us SLOWER — the extra 512-col streams hold the PE's HAM
  activity monitor at 2.4 GHz through VectorE supply gaps (default PE
  state is K=4/8 half-clock; >~5 us idle re-throttles).
"""

import numpy as np

B, D_IN, D_OUT, N_BANK, N_CORES = 256, 512, 512, 1000, 8
P = 128  # partitions
C = D_IN // P  # 4 k-chunks
M = 4  # samples per bank slot (max observed bank multiplicity is 3)
NWARM = 10  # PE warm-up matmuls (HAM throttle release)

_CACHE = {}
last_exec_time_ns = None


def _build_nc(K, zero_bias):
    """Device graph for K bank-slots per core (K even; padded on host).

    zero_bias=True omits the membias path entirely (bias input is all
    zeros, as produced by the reference setup); the general graph keeps
    it via the per-slot k=1 bias matmul."""
    import concourse.mybir as mybir
    import concourse.tile as tile
    from concourse import bacc

    f32 = mybir.dt.float32
    bf16 = mybir.dt.bfloat16
    nc = bacc.Bacc()

    R = K * M  # output rows, slot-major: row t*M + j = slot t, column j
    K2 = K // 2  # load pairs
    W2 = 2 * C * D_OUT  # free elems per pair tile (2 slots)
    # eg2[q, p, u*2048 + c*512 + o] = E[2q+u, c*128+p, o] (bf16, host layout)
    eg2 = nc.dram_tensor("eg2", [K2, P, W2], bf16, kind="ExternalInput")
    wt = nc.dram_tensor("wt", [P, C * D_OUT], bf16, kind="ExternalInput")
    xtt = nc.dram_tensor("xtt", [P, C * R], bf16, kind="ExternalInput")
    if not zero_bias:
        bb = nc.dram_tensor("bb", [K, D_OUT], f32, kind="ExternalInput")
        beg = nc.dram_tensor("beg", [K, D_OUT], f32, kind="ExternalInput")
    out = nc.dram_tensor("out", [R, D_OUT], f32, kind="ExternalOutput")

    with tile.TileContext(nc) as tc:
        with (
            tc.tile_pool(name="const", bufs=1) as constp,
            tc.tile_pool(name="ep", bufs=6) as ep,
            tc.tile_pool(name="wep", bufs=4) as wep,
            tc.tile_pool(name="ps", bufs=3, space="PSUM") as psp,
            tc.tile_pool(name="scr", bufs=2) as scr,
        ):
            # Resident operands (already bf16 from host; scalar=Activation
            # HWDGE ring carries the small loads, W first so the multiply
            # pipeline can start as soon as the first bank pair lands).
            w_b = constp.tile([P, C * D_OUT], bf16, name="wb")
            nc.scalar.dma_start(w_b[:], wt[:])
            x_b = constp.tile([P, C * R], bf16)
            nc.scalar.dma_start(x_b[:], xtt[:])

            if not zero_bias:
                # membias = bias * Berr[bank], one row per bank slot, in
                # bf16: it joins the PE accumulation via a k=1 matmul below,
                # which needs the rhs at partition 0 (single reshape DMA).
                bias_k = scr.tile([K, D_OUT], f32, name="bias_k", tag="bq")
                nc.scalar.dma_start(bias_k[:], bb[:])
                berr_k = scr.tile([K, D_OUT], f32, name="berr_k", tag="eq")
                nc.scalar.dma_start(berr_k[:], beg[:])
                # mbk runs on VectorE (first in its queue, ~0.9 us): on
                # gpsimd it starts ~18 us in, and the mbrow DMA below blocks
                # the scalar engine queue — and with it every scalar-ring
                # pair load — until mbk completes.
                mbk = constp.tile([K, D_OUT], bf16, name="mbk")
                nc.vector.tensor_mul(mbk[:], bias_k[:], berr_k[:])
                mbrow = constp.tile([1, K * D_OUT], bf16)
                nc.scalar.dma_start(mbrow[:], mbk[:])
                ones_b = constp.tile([1, M], bf16)
                nc.any.memset(ones_b[:], 1.0)

            warm = psp.tile([M, D_OUT], f32, name="warm", bufs=1)

            # Dummy matmuls on resident tiles release the PE's HAM throttle
            # (default state is K=4/8 half-clock; ~3.4us of activity frees it)
            for _ in range(NWARM):
                nc.tensor.matmul(
                    warm[:], x_b[:, 0:M], w_b[:, 0:D_OUT], start=True, stop=True
                )

            # output staging at partition base 0: osb[j, t*512+n] = Z[t*M+j, n]
            osb = constp.tile([M, K * D_OUT], f32, name="osb")
            # alternate the 1 MB pair loads over the two HWDGE rings (the
            # gpsimd SWDGE ring issues loads ~6us apart — too slow to help)
            def load_ring(q):
                return (nc.sync, nc.scalar)[q % 2]

            H = C * D_OUT
            for q in range(K2):
                ebp = ep.tile([P, W2], bf16)
                if q == 0:
                    # first pair split in two halves (both on the sync ring:
                    # a half on the scalar ring delays that whole queue) so
                    # the first multiply's region dep fires ~1.4us after the
                    # first half lands instead of after the full 1 MB
                    nc.sync.dma_start(ebp[:, 0:H], eg2[0, :, 0:H])
                    nc.sync.dma_start(ebp[:, H:W2], eg2[0, :, H:W2])
                else:
                    load_ring(q).dma_start(ebp[:], eg2[q])
                wep_t = wep.tile([P, W2], bf16)
                ps = psp.tile([M, 2 * D_OUT], f32)  # slot pair: 2 PSUM banks
                for u in range(2):
                    t = 2 * q + u
                    nc.vector.tensor_mul(
                        wep_t[:, u * C * D_OUT : (u + 1) * C * D_OUT],
                        ebp[:, u * C * D_OUT : (u + 1) * C * D_OUT],
                        w_b[:],
                    )
                    for c in range(C):
                        nc.tensor.matmul(
                            ps[:, u * D_OUT : (u + 1) * D_OUT],
                            x_b[:, (c * K + t) * M : (c * K + t) * M + M],
                            wep_t[
                                :,
                                u * C * D_OUT + c * D_OUT : u * C * D_OUT
                                + (c + 1) * D_OUT,
                            ],
                            start=(c == 0),
                            stop=(zero_bias and c == C - 1),
                        )
                    if not zero_bias:
                        # bias joins the PSUM accumulation: the k=1 matmul
                        # ones[1,M]^T @ mb[1,N] broadcasts the bank's membias
                        # row onto all M output rows
                        nc.tensor.matmul(
                            ps[:, u * D_OUT : (u + 1) * D_OUT],
                            ones_b[:],
                            mbrow[0:1, t * D_OUT : (t + 1) * D_OUT],
                            start=False,
                            stop=True,
                        )
                # drain the pair on the otherwise idle ScalarE into the
                # staging tile; one store at the end
                nc.scalar.copy(
                    osb[0:M, 2 * q * D_OUT : 2 * (q + 1) * D_OUT], ps[:]
                )

            nc.sync.dma_start(
                out[:].rearrange("(t j) n -> j t n", j=M),
                osb[:].rearrange("j (t n) -> j t n", n=D_OUT),
            )

    nc.compile()
    return nc


def _pack(idx):
    """Group samples by bank, pack banks onto cores.

    Returns (K, plan) where plan[c] is a list of (bank, [samples]) slots,
    each slot carrying at most M samples of one bank. K is even.
    """
    from collections import defaultdict

    groups = defaultdict(list)
    for s, b in enumerate(idx):
        groups[int(b)].append(s)
    # one slot per <=M samples of a bank
    slots = []
    for b, ss in groups.items():
        for i in range(0, len(ss), M):
            slots.append((b, ss[i : i + M]))
    slots.sort(key=lambda x: -len(x[1]))
    plan = [[] for _ in range(N_CORES)]
    for b, ss in slots:
        c = min(range(N_CORES), key=lambda c: len(plan[c]))
        plan[c].append((b, ss))
    K = max(len(p) for p in plan)
    K += K % 2  # pair loads need an even slot count
    return K, plan


def _install_trace_shim():
    """Register the axon NTFF profile hook bass_utils expects (the agent
    image lacks antenv.axon_hooks; the C ABI is in libaxon_pjrt.so)."""
    import contextlib
    import ctypes
    import sys
    import types

    if "antenv.axon_hooks" in sys.modules:
        return
    lib = ctypes.CDLL("/opt/axon/libaxon_pjrt.so")
    if not hasattr(lib, "axon_start_nrt_profile"):
        hook = None
    else:
        lib.axon_start_nrt_profile.argtypes = [
            ctypes.POINTER(ctypes.c_int64),
            ctypes.c_size_t,
        ]
        lib.axon_start_nrt_profile.restype = ctypes.c_int64
        lib.axon_stop_nrt_profile.argtypes = [ctypes.c_char_p]
        lib.axon_stop_nrt_profile.restype = ctypes.c_int64

        @contextlib.contextmanager
        def hook(output_dir, device_ids):
            import jax

            jax.devices()
            if device_ids:
                ids = (ctypes.c_int64 * len(device_ids))(*device_ids)
                rc = lib.axon_start_nrt_profile(ids, len(device_ids))
            else:
                rc = lib.axon_start_nrt_profile(None, 0)
            if rc != 0:
                raise RuntimeError(f"axon_start_nrt_profile rc={rc}")
            try:
                yield
            finally:
                n = lib.axon_stop_nrt_profile(str(output_dir).encode())
                print(f"ntff profile: {n} file(s) -> {output_dir}", file=sys.stderr)

    mod = types.ModuleType("antenv.axon_hooks")
    mod.get_axon_ntff_profile_hook = lambda: hook
    mod.set_axon_ntff_profile_hook = lambda h: None
    sys.modules["antenv.axon_hooks"] = mod


def kernel(X, W, bias, Werr_bank, Berr_bank, idx):
    global last_exec_time_ns
    import os

    import ml_dtypes

    from concourse.bass_utils import run_bass_kernel_spmd

    bf16 = ml_dtypes.bfloat16
    X = np.asarray(X, dtype=np.float32)
    W = np.asarray(W, dtype=np.float32)
    bias = np.asarray(bias, dtype=np.float32)
    Werr_bank = np.asarray(Werr_bank, dtype=np.float32)
    Berr_bank = np.asarray(Berr_bank, dtype=np.float32)
    idx = np.asarray(idx, dtype=np.int32)

    K, plan = _pack(idx)
    # NB: a zero-bias specialization (dropping the per-slot bias matmul)
    # measures ~5us SLOWER despite less PE work — the 512-col bias matmuls
    # double as HAM activity keepers that hold the PE at 2.4 GHz through
    # VectorE supply gaps. Keep the bias path unconditionally.
    zero_bias = False
    if ("nc", K, zero_bias) not in _CACHE:
        _CACHE[("nc", K, zero_bias)] = _build_nc(K, zero_bias)
    nc = _CACHE[("nc", K, zero_bias)]
    R = K * M
    K2 = K // 2

    # Host-side sharding / layout (pure data movement + dtype cast).
    wt = np.ascontiguousarray(
        W.astype(bf16).reshape(C, P, D_OUT).transpose(1, 0, 2).reshape(P, C * D_OUT)
    )
    bb = np.ascontiguousarray(np.broadcast_to(bias.reshape(1, D_OUT), (K, D_OUT)))

    in_maps = []
    row_of_sample = np.full(B, -1, dtype=np.int64)  # (core, row) flattened
    for c_id in range(N_CORES):
        slots = plan[c_id]
        banks = [b for b, _ in slots] + [0] * (K - len(slots))
        eg = Werr_bank[banks].astype(bf16)  # [K, D_in, D_out] bf16
        # pair-interleaved layout: eg2[q, p, u*2048 + c*512 + o]
        eg2 = np.ascontiguousarray(
            eg.reshape(K2, 2, C, P, D_OUT)
            .transpose(0, 3, 1, 2, 4)
            .reshape(K2, P, 2 * C * D_OUT)
        )
        # X columns and output rows in slot-major order: row t*M + j
        xs = np.zeros((R, D_IN), dtype=np.float32)
        beg = np.ascontiguousarray(Berr_bank[banks, 0, :])  # [K, D_out]
        for t, (b, ss) in enumerate(slots):
            for j, s in enumerate(ss):
                xs[t * M + j] = X[s]
                row_of_sample[s] = c_id * R + t * M + j
        xtt = np.ascontiguousarray(
            xs.T.astype(bf16).reshape(C, P, R).transpose(1, 0, 2).reshape(P, C * R)
        )
        m = {"eg2": eg2, "wt": wt, "xtt": xtt}
        if not zero_bias:
            m["bb"] = bb
            m["beg"] = beg
        in_maps.append(m)
    assert (row_of_sample >= 0).all()

    trace = os.environ.get("BASS_KERNEL_TRACE") == "1"
    if trace:
        _install_trace_shim()
    res = run_bass_kernel_spmd(
        nc,
        in_maps,
        core_ids=list(range(N_CORES)),
        trace=trace,
        trace_cores=(
            list(range(N_CORES))
            if os.environ.get("BASS_KERNEL_TRACE_ALL") == "1"
            else [0]
        )
        if trace
        else None,
    )
    last_exec_time_ns = res.exec_time_ns
    allrows = np.concatenate([r["out"] for r in res.results], axis=0)  # [8*R, 512]
    return np.ascontiguousarray(allrows[row_of_sample])
